# revision 9
# baseline (speedup 1.0000x reference)
"""MCWAUCHLoss Trainium2 kernel.

Shards the [B, C] = [65536, 256] inputs row-wise across 8 NeuronCores
(8192 rows each). Inputs are cast to bf16 on host (labels are exactly
representable; x rounding washes out across the >=8k-element reductions),
halving DMA traffic and enabling DVE 2x perf modes.

Per core (phase A = sigmoid table set, phase B = natural_log set):
  phase A, per tile t:  s_t = sigmoid(x_t)            (ACT)
                        acc_lx[t]  = sum lab*x        (DVE scalar_tensor_tensor)
                        acc_x[t]   = sum x            (DVE stt bypass)
                        PSUM nlab += ones^T @ lab     (PE, per-category)
  phase B, per tile t:  ls_t = ln(s_t), acc_ls[t] = sum ls   (ACT + accum_out)
                        acc_lls[t] = sum lab*ls       (DVE stt)
                        lt_t = lab*s_t                (POOL)
                        PSUM s += ones^T @ s, PSUM lt += ones^T @ lt  (PE)

Host combine (f64) using log(sigmoid(x)) = ls, log(1-sigmoid(x)) = ls - x:
  PL = sum lab*ls                      = L_ls
  NL = sum (1-lab)*(ls-x)              = (S_ls - L_ls) - (S_x - L_x)
plus the per-category AUC hinge from n_pos / sum_s / sum_pos.
"""

import sys

import numpy as np

sys.path.insert(0, "/opt/trn_rl_repo")

from contextlib import ExitStack

def _ensure_axon_hooks():
    """Provide antenv.axon_hooks if the image lacks it (needed only when
    profiling with trace=True; harmless otherwise)."""
    try:
        import antenv.axon_hooks  # noqa: F401
        return
    except ImportError:
        pass
    import types

    try:
        import antenv
    except ImportError:
        return
    mod = types.ModuleType("antenv.axon_hooks")
    mod._HOOK = None

    def set_axon_ntff_profile_hook(h):
        mod._HOOK = h

    def get_axon_ntff_profile_hook():
        if mod._HOOK is None:
            try:
                from trn_agent_boot.trn_boot import _ntff_profile_via_ctypes

                mod._HOOK = _ntff_profile_via_ctypes("/opt/axon/libaxon_pjrt.so")
            except Exception:
                return None
        return mod._HOOK

    mod.set_axon_ntff_profile_hook = set_axon_ntff_profile_hook
    mod.get_axon_ntff_profile_hook = get_axon_ntff_profile_hook
    sys.modules["antenv.axon_hooks"] = mod
    antenv.axon_hooks = mod


_ensure_axon_hooks()

import ml_dtypes
import concourse.bacc as bacc
import concourse.tile as tile
from concourse import mybir
from concourse.tile import add_dep_helper
from concourse.bass_utils import run_bass_kernel_spmd

B, C = 65536, 256
N_CORES = 8
R = B // N_CORES            # 8192 rows per core
TILE_ROWS = 2048            # rows per SBUF tile
T = R // TILE_ROWS          # 4 tiles per core
P = 128                     # partitions
RG = TILE_ROWS // P         # 16 rowgroups per tile
FREE = RG * C               # 4096 free elements per partition
MM_N = 512                  # matmul moving free dim (2 rowgroups worth)
MM_PER_TILE = FREE // MM_N  # 8

BF = mybir.dt.bfloat16
F32 = mybir.dt.float32

_PROGRAM = None


def _build_program():
    nc = bacc.Bacc("TRN2", target_bir_lowering=False, debug=False)

    x_d = nc.dram_tensor("x", [R, C], BF, kind="ExternalInput").ap()
    lab_d = nc.dram_tensor("lab", [R, C], BF, kind="ExternalInput").ap()
    # rows: 0 = n_pos, 1 = sum x, 2 = sum s, 3 = sum lab*s   (col j: category
    # j%256, even/odd rowgroup half j//256)
    o_cat = nc.dram_tensor("o_cat", [4, MM_N], F32, kind="ExternalOutput").ap()
    # rows: 0 = sum ls, 1 = sum lab*ls, 2 = sum lab*x, 3 = sum x  (per
    # partition, col = tile)
    o_acc = nc.dram_tensor("o_acc", [4, P, T], F32, kind="ExternalOutput").ap()

    with tile.TileContext(nc) as tc, ExitStack() as ctx:
        const = ctx.enter_context(tc.tile_pool(name="const", bufs=1))
        labp = ctx.enter_context(tc.tile_pool(name="labp", bufs=1))
        sp = ctx.enter_context(tc.tile_pool(name="sp", bufs=1))
        xp = ctx.enter_context(tc.tile_pool(name="xp", bufs=3))
        work = ctx.enter_context(tc.tile_pool(name="work", bufs=2))
        junkp = ctx.enter_context(tc.tile_pool(name="junkp", bufs=1))
        accp = ctx.enter_context(tc.tile_pool(name="accp", bufs=1))
        psum = ctx.enter_context(tc.tile_pool(name="psum", bufs=1, space="PSUM"))

        ones = const.tile([P, 1], BF, tag="ones")
        nc.vector.memset(ones, 1.0)

        acc_ls = accp.tile([P, T], F32, tag="acc_ls")
        acc_lls = accp.tile([P, T], F32, tag="acc_lls")
        acc_lx = accp.tile([P, T], F32, tag="acc_lx")
        acc_x = accp.tile([P, T], F32, tag="acc_x")

        ps_nlab = psum.tile([1, MM_N], F32, tag="ps_nlab")
        ps_x = psum.tile([1, MM_N], F32, tag="ps_x")
        ps_s = psum.tile([1, MM_N], F32, tag="ps_s")
        ps_lt = psum.tile([1, MM_N], F32, tag="ps_lt")

        junk = junkp.tile([P, FREE], BF, tag="junk")

        mul = mybir.AluOpType.mult
        add = mybir.AluOpType.add
        byp = mybir.AluOpType.bypass

        # --- phase A: sigmoid table set ---
        acts_a = []
        st = []
        lt_tiles = []
        for t in range(T):
            rows = slice(t * TILE_ROWS, (t + 1) * TILE_ROWS)
            xt = xp.tile([P, FREE], BF, tag="x")
            nc.sync.dma_start(
                out=xt, in_=x_d[rows, :].rearrange("(p r) c -> p (r c)", p=P)
            )
            lt = labp.tile([P, FREE], BF, tag=f"lab{t}")
            nc.sync.dma_start(
                out=lt, in_=lab_d[rows, :].rearrange("(p r) c -> p (r c)", p=P)
            )
            lt_tiles.append(lt)

            s = sp.tile([P, FREE], BF, tag=f"s{t}")
            ia = nc.scalar.activation(
                out=s, in_=xt, func=mybir.ActivationFunctionType.Sigmoid
            )
            acts_a.append(ia)
            st.append(s)

            nc.vector.scalar_tensor_tensor(
                out=junk, in0=lt, scalar=1.0, in1=xt,
                op0=mul, op1=mul, accum_out=acc_lx[:, t : t + 1],
            )
            nc.vector.scalar_tensor_tensor(
                out=junk, in0=xt, scalar=1.0, in1=xt,
                op0=mul, op1=byp, accum_out=acc_x[:, t : t + 1],
            )
            for k in range(MM_PER_TILE):
                first = t == 0 and k == 0
                last = t == T - 1 and k == MM_PER_TILE - 1
                sl = slice(k * MM_N, (k + 1) * MM_N)
                nc.tensor.matmul(ps_nlab, ones, lt[:, sl], start=first, stop=last)
                nc.tensor.matmul(ps_x, ones, xt[:, sl], start=first, stop=last)

        # --- phase B: natural_log table set ---
        acts_b = []
        for t in range(T):
            s = st[t]
            lab = lt_tiles[t]
            ls = work.tile([P, FREE], BF, tag="ls")
            ib = nc.scalar.activation(
                out=ls,
                in_=s,
                func=mybir.ActivationFunctionType.Ln,
                accum_out=acc_ls[:, t : t + 1],
            )
            acts_b.append(ib)
            nc.vector.scalar_tensor_tensor(
                out=junk, in0=lab, scalar=1.0, in1=ls,
                op0=mul, op1=mul, accum_out=acc_lls[:, t : t + 1],
            )
            lt = work.tile([P, FREE], BF, tag="lt")
            nc.gpsimd.tensor_mul(lt, lab, s)
            for k in range(MM_PER_TILE):
                first = t == 0 and k == 0
                last = t == T - 1 and k == MM_PER_TILE - 1
                sl = slice(k * MM_N, (k + 1) * MM_N)
                nc.tensor.matmul(ps_s, ones, s[:, sl], start=first, stop=last)
                nc.tensor.matmul(ps_lt, ones, lt[:, sl], start=first, stop=last)

        # keep the ACT engine phase-ordered: each table set loads exactly once
        for ia in acts_a:
            for ib in acts_b:
                # first arg waits on second: every Ln runs after every Sigmoid
                add_dep_helper(
                    ib.ins, ia.ins, sync=False, reason="act table phase order"
                )

        # --- outputs (PSUM staged through SBUF; engine writes must start
        # at partition 0, so one [1, N] tile per quantity) ---
        for i, ps in enumerate([ps_nlab, ps_x, ps_s, ps_lt]):
            cat_sb = accp.tile([1, MM_N], F32, tag=f"cat_sb{i}")
            nc.vector.tensor_copy(cat_sb, ps)
            nc.sync.dma_start(out=o_cat[i : i + 1, :], in_=cat_sb)
        nc.sync.dma_start(out=o_acc[0], in_=acc_ls)
        nc.sync.dma_start(out=o_acc[1], in_=acc_lls)
        nc.sync.dma_start(out=o_acc[2], in_=acc_lx)
        nc.sync.dma_start(out=o_acc[3], in_=acc_x)

    nc.compile()
    return nc


def _get_program():
    global _PROGRAM
    if _PROGRAM is None:
        _PROGRAM = _build_program()
    return _PROGRAM


def _run_on_hw(x, lab, **kwargs):
    nc = _get_program()
    xb = np.asarray(x, dtype=np.float32).astype(ml_dtypes.bfloat16)
    lb = np.asarray(lab, dtype=np.float32).astype(ml_dtypes.bfloat16)
    in_maps = []
    for m in range(N_CORES):
        rows = slice(m * R, (m + 1) * R)
        in_maps.append(
            {
                "x": np.ascontiguousarray(xb[rows]),
                "lab": np.ascontiguousarray(lb[rows]),
            }
        )
    return run_bass_kernel_spmd(nc, in_maps, core_ids=list(range(N_CORES)), **kwargs)


def _combine(results):
    n_pos = np.zeros(C, np.float64)
    S_x_cat = np.zeros(C, np.float64)
    sum_s = np.zeros(C, np.float64)
    sum_pos = np.zeros(C, np.float64)
    S_ls = 0.0
    L_ls = 0.0
    L_x = 0.0
    for r in results:
        cat = r["o_cat"].astype(np.float64)
        n_pos += cat[0, :C] + cat[0, C:]
        S_x_cat += cat[1, :C] + cat[1, C:]
        sum_s += cat[2, :C] + cat[2, C:]
        sum_pos += cat[3, :C] + cat[3, C:]
        acc = r["o_acc"].astype(np.float64)
        S_ls += acc[0].sum()
        L_ls += acc[1].sum()
        L_x += acc[2].sum()

    S_x = S_x_cat.sum()
    total = float(B) * float(C)
    PL = L_ls
    NL = (S_ls - L_ls) - (S_x - L_x)
    num_P = n_pos.sum()
    alpha_P = num_P / total
    alpha_N = (total - num_P) / total
    cel = -alpha_N * (PL / total) - alpha_P * (NL / total)

    n_neg = float(B) - n_pos
    mean_pos = sum_pos / np.maximum(n_pos, 1.0)
    mean_neg = (sum_s - sum_pos) / np.maximum(n_neg, 1.0)
    both = (n_pos > 0) & (n_neg > 0)
    pen = np.where(
        both,
        1.0 - mean_pos + mean_neg,
        np.where(n_pos == 0, 1.0 + mean_neg, 1.0 - mean_pos),
    )
    cls = cel + 0.1 * (pen.sum() / C)
    return (np.float32(cls), np.float32(0.1 * pen[-1]))


def kernel(output, labels):
    res = _run_on_hw(output, labels)
    return _combine(res.results)


if __name__ == "__main__":
    x = np.random.randn(B, C).astype(np.float32)
    lab = (np.random.rand(B, C) < 0.3).astype(np.float32)
    print(kernel(output=x, labels=lab))


# revision 10
# speedup vs baseline: 1.4676x; 1.4676x over previous
"""MCWAUCHLoss Trainium2 kernel.

Shards the [B, C] = [65536, 256] inputs row-wise across 8 NeuronCores
(8192 rows each). Inputs are cast to bf16 on host (labels exactly
representable; x rounding washes out across the >=8k-element reductions).

Per core, per tile (phase A = sigmoid table set, phase B = natural_log):
  A:  s    = sigmoid(x)                (ACT)
      labc = 1 - lab                   (DVE tensor_scalar)
      lt   = lab * s                   (DVE)
      w1   = lt + labc                 (DVE)   -> s where lab=1 else 1 (exact)
      d    = s - lt                    (DVE)   -> s where lab=0 else 0 (exact)
      PSUM s  += ones^T @ s            (PE, per-category)
      PSUM lt += ones^T @ lt           (PE, per-category)
  B:  acc_pl[t] = sum ln(w1)           (ACT accum_out)  = sum lab*ln(s)
      acc_nl[t] = sum ln(1 - d)        (ACT accum_out, scale=-1 bias=1)
                                       = sum (1-lab)*ln(1-s)
ln(1) = 0 makes the masking exact. n_pos comes from a host-side
labels.sum(0); no x-only reductions are needed on device.
"""

import sys

import numpy as np

sys.path.insert(0, "/opt/trn_rl_repo")

from contextlib import ExitStack


def _ensure_axon_hooks():
    """Provide antenv.axon_hooks if the image lacks it (needed only when
    profiling with trace=True; harmless otherwise)."""
    try:
        import antenv.axon_hooks  # noqa: F401
        return
    except ImportError:
        pass
    import types

    try:
        import antenv
    except ImportError:
        return
    mod = types.ModuleType("antenv.axon_hooks")
    mod._HOOK = None

    def set_axon_ntff_profile_hook(h):
        mod._HOOK = h

    def get_axon_ntff_profile_hook():
        if mod._HOOK is None:
            try:
                from trn_agent_boot.trn_boot import _ntff_profile_via_ctypes

                mod._HOOK = _ntff_profile_via_ctypes("/opt/axon/libaxon_pjrt.so")
            except Exception:
                return None
        return mod._HOOK

    mod.set_axon_ntff_profile_hook = set_axon_ntff_profile_hook
    mod.get_axon_ntff_profile_hook = get_axon_ntff_profile_hook
    sys.modules["antenv.axon_hooks"] = mod
    antenv.axon_hooks = mod


_ensure_axon_hooks()

import ml_dtypes
import concourse.bacc as bacc
import concourse.tile as tile
from concourse import mybir
from concourse.tile import add_dep_helper
from concourse.bass_utils import run_bass_kernel_spmd

B, C = 65536, 256
N_CORES = 8
R = B // N_CORES            # 8192 rows per core
TILE_ROWS = 2048            # rows per SBUF tile
T = R // TILE_ROWS          # 4 tiles per core
P = 128                     # partitions
RG = TILE_ROWS // P         # 16 rowgroups per tile
FREE = RG * C               # 4096 free elements per partition
MM_N = 512                  # matmul moving free dim (2 rowgroups worth)
MM_PER_TILE = FREE // MM_N  # 8

BF = mybir.dt.bfloat16
F32 = mybir.dt.float32

_PROGRAM = None


def _build_program():
    nc = bacc.Bacc("TRN2", target_bir_lowering=False, debug=False)

    x_d = nc.dram_tensor("x", [R, C], BF, kind="ExternalInput").ap()
    lab_d = nc.dram_tensor("lab", [R, C], BF, kind="ExternalInput").ap()
    # rows: 0 = sum s, 1 = sum lab*s   (col j: category j%256, even/odd
    # rowgroup half j//256)
    o_cat = nc.dram_tensor("o_cat", [2, MM_N], F32, kind="ExternalOutput").ap()
    # rows: 0 = sum ln(w1) = PL part, 1 = sum ln(1-d) = NL part
    o_acc = nc.dram_tensor("o_acc", [2, P, T], F32, kind="ExternalOutput").ap()

    with tile.TileContext(nc) as tc, ExitStack() as ctx:
        const = ctx.enter_context(tc.tile_pool(name="const", bufs=1))
        xp = ctx.enter_context(tc.tile_pool(name="xp", bufs=3))
        labp = ctx.enter_context(tc.tile_pool(name="labp", bufs=3))
        sp = ctx.enter_context(tc.tile_pool(name="sp", bufs=2))
        wp = ctx.enter_context(tc.tile_pool(name="wp", bufs=1))
        work = ctx.enter_context(tc.tile_pool(name="work", bufs=2))
        junkp = ctx.enter_context(tc.tile_pool(name="junkp", bufs=1))
        accp = ctx.enter_context(tc.tile_pool(name="accp", bufs=1))
        psum = ctx.enter_context(tc.tile_pool(name="psum", bufs=1, space="PSUM"))

        ones = const.tile([P, 1], BF, tag="ones")
        nc.vector.memset(ones, 1.0)

        acc_pl = accp.tile([P, T], F32, tag="acc_pl")
        acc_nl = accp.tile([P, T], F32, tag="acc_nl")

        ps_s = psum.tile([1, MM_N], F32, tag="ps_s")
        ps_lt = psum.tile([1, MM_N], F32, tag="ps_lt")

        junk = junkp.tile([P, FREE], BF, tag="junk")

        mul = mybir.AluOpType.mult
        add = mybir.AluOpType.add
        sub = mybir.AluOpType.subtract

        # --- phase A: sigmoid table set + all DVE products + PE ---
        acts_a = []
        w1t = []
        dt_ = []
        for t in range(T):
            rows = slice(t * TILE_ROWS, (t + 1) * TILE_ROWS)
            xt = xp.tile([P, FREE], BF, tag="x")
            nc.sync.dma_start(
                out=xt, in_=x_d[rows, :].rearrange("(p r) c -> p (r c)", p=P)
            )
            lab = labp.tile([P, FREE], BF, tag="lab")
            nc.sync.dma_start(
                out=lab, in_=lab_d[rows, :].rearrange("(p r) c -> p (r c)", p=P)
            )

            s = sp.tile([P, FREE], BF, tag="s")
            ia = nc.scalar.activation(
                out=s, in_=xt, func=mybir.ActivationFunctionType.Sigmoid
            )
            acts_a.append(ia)

            labc = work.tile([P, FREE], BF, tag="labc")
            nc.vector.tensor_scalar(
                out=labc, in0=lab, scalar1=-1.0, scalar2=1.0, op0=mul, op1=add
            )
            lt = work.tile([P, FREE], BF, tag="lt")
            nc.vector.tensor_mul(lt, lab, s)
            w1 = wp.tile([P, FREE], BF, tag=f"w1_{t}")
            nc.vector.tensor_tensor(out=w1, in0=lt, in1=labc, op=add)
            d = wp.tile([P, FREE], BF, tag=f"d_{t}")
            nc.vector.tensor_tensor(out=d, in0=s, in1=lt, op=sub)
            w1t.append(w1)
            dt_.append(d)

            for k in range(MM_PER_TILE):
                first = t == 0 and k == 0
                last = t == T - 1 and k == MM_PER_TILE - 1
                sl = slice(k * MM_N, (k + 1) * MM_N)
                nc.tensor.matmul(ps_s, ones, s[:, sl], start=first, stop=last)
                nc.tensor.matmul(ps_lt, ones, lt[:, sl], start=first, stop=last)

        # --- phase B: natural_log table set, accumulating scalar sums ---
        acts_b = []
        for t in range(T):
            ib = nc.scalar.activation(
                out=junk,
                in_=w1t[t],
                func=mybir.ActivationFunctionType.Ln,
                accum_out=acc_pl[:, t : t + 1],
            )
            acts_b.append(ib)
            ib2 = nc.scalar.activation(
                out=junk,
                in_=dt_[t],
                func=mybir.ActivationFunctionType.Ln,
                scale=-1.0,
                bias=1.0,
                accum_out=acc_nl[:, t : t + 1],
            )
            acts_b.append(ib2)

        # keep the ACT engine phase-ordered: each table set loads exactly once
        for ia in acts_a:
            for ib in acts_b:
                # first arg waits on second: every Ln runs after every Sigmoid
                add_dep_helper(
                    ib.ins, ia.ins, sync=False, reason="act table phase order"
                )

        # --- outputs (PSUM staged through SBUF; engine writes must start
        # at partition 0, so one [1, N] tile per quantity) ---
        for i, ps in enumerate([ps_s, ps_lt]):
            cat_sb = accp.tile([1, MM_N], F32, tag=f"cat_sb{i}")
            nc.vector.tensor_copy(cat_sb, ps)
            nc.sync.dma_start(out=o_cat[i : i + 1, :], in_=cat_sb)
        nc.sync.dma_start(out=o_acc[0], in_=acc_pl)
        nc.sync.dma_start(out=o_acc[1], in_=acc_nl)

    nc.compile()
    return nc


def _get_program():
    global _PROGRAM
    if _PROGRAM is None:
        _PROGRAM = _build_program()
    return _PROGRAM


def _run_on_hw(x, lab, **kwargs):
    nc = _get_program()
    xb = np.asarray(x, dtype=np.float32).astype(ml_dtypes.bfloat16)
    lb = np.asarray(lab, dtype=np.float32).astype(ml_dtypes.bfloat16)
    in_maps = []
    for m in range(N_CORES):
        rows = slice(m * R, (m + 1) * R)
        in_maps.append(
            {
                "x": np.ascontiguousarray(xb[rows]),
                "lab": np.ascontiguousarray(lb[rows]),
            }
        )
    return run_bass_kernel_spmd(nc, in_maps, core_ids=list(range(N_CORES)), **kwargs)


def _combine(results, labels):
    sum_s = np.zeros(C, np.float64)
    sum_pos = np.zeros(C, np.float64)
    PL = 0.0
    NL = 0.0
    for r in results:
        cat = r["o_cat"].astype(np.float64)
        sum_s += cat[0, :C] + cat[0, C:]
        sum_pos += cat[1, :C] + cat[1, C:]
        acc = r["o_acc"].astype(np.float64)
        PL += acc[0].sum()
        NL += acc[1].sum()

    n_pos = labels.sum(axis=0, dtype=np.float64)
    total = float(B) * float(C)
    num_P = n_pos.sum()
    alpha_P = num_P / total
    alpha_N = (total - num_P) / total
    cel = -alpha_N * (PL / total) - alpha_P * (NL / total)

    n_neg = float(B) - n_pos
    mean_pos = sum_pos / np.maximum(n_pos, 1.0)
    mean_neg = (sum_s - sum_pos) / np.maximum(n_neg, 1.0)
    both = (n_pos > 0) & (n_neg > 0)
    pen = np.where(
        both,
        1.0 - mean_pos + mean_neg,
        np.where(n_pos == 0, 1.0 + mean_neg, 1.0 - mean_pos),
    )
    cls = cel + 0.1 * (pen.sum() / C)
    return (np.float32(cls), np.float32(0.1 * pen[-1]))


def kernel(output, labels):
    res = _run_on_hw(output, labels)
    return _combine(res.results, np.asarray(labels))


if __name__ == "__main__":
    x = np.random.randn(B, C).astype(np.float32)
    lab = (np.random.rand(B, C) < 0.3).astype(np.float32)
    print(kernel(output=x, labels=lab))
